# revision 10
# baseline (speedup 1.0000x reference)
"""Causal self-attention on 8 trn2 NeuronCores.

Sharding: core c handles batch b = c // 4 and head group g = c % 4
(heads 4g..4g+3 of 16).  Each core computes:
  stage A: qkT = (W_qk_slice)^T @ x^T   (feature-major, d-major q/k)
           v   = x @ W_v_slice          (token-major, + ones column)
  stage B: per head, causal attention in S^T layout (keys on partitions,
           q on free dim): S^T = k @ q^T, P = exp(S/8) * mask,
           pv = [v | 1]^T @ P^T  -> rows 0..63 = out^T, row 64 = denom
           z = out^T / denom  (feature-major attention output)
  stage C: y_partial = z^T @ W_proj[row slice]   (token-major)
Host sums the 4 partials per batch and adds b_proj.

All matmuls run as float32r (TF32-like, full rate at N>=256).
"""

import numpy as np

B, T, C, H, D = 2, 2048, 1024, 16, 64
HPC = 4              # heads per core
FW = HPC * D         # 256 attention-output features per core
QKF = 2 * FW         # 512 q+k features per core
NTW = T // 512       # 4 q/token windows of 512
NTT = T // 128       # 16 token tiles of 128
NKC = C // 128       # 8 contraction chunks for stage A

_CACHE = {}


def _build_nc():
    import concourse.bass as bass  # noqa: F401
    import concourse.mybir as mybir
    import concourse.tile as tile
    from concourse import bacc
    from contextlib import ExitStack

    f32 = mybir.dt.float32
    r32 = mybir.dt.float32r
    AF = mybir.ActivationFunctionType

    nc = bacc.Bacc(None, target_bir_lowering=False)
    xT = nc.declare_dram_parameter("xT", [C, T], r32, isOutput=False)
    w_qk = nc.declare_dram_parameter("w_qk", [C, QKF], r32, isOutput=False)
    b_qk = nc.declare_dram_parameter("b_qk", [QKF], f32, isOutput=False)
    w_v = nc.declare_dram_parameter("w_v", [C, FW], r32, isOutput=False)
    b_v = nc.declare_dram_parameter("b_v", [FW], r32, isOutput=False)
    w_p = nc.declare_dram_parameter("w_p", [FW, C], r32, isOutput=False)
    masks = nc.declare_dram_parameter("masks", [4, 128, 512], r32, isOutput=False)
    y = nc.declare_dram_parameter("y", [T, C], f32, isOutput=True)

    with nc.allow_low_precision(reason="fp32r matmul dataflow"), \
            tile.TileContext(nc) as tc, ExitStack() as ctx:
        wpool = ctx.enter_context(tc.tile_pool(name="wpool", bufs=1))
        big = ctx.enter_context(tc.tile_pool(name="big", bufs=1))
        xw = ctx.enter_context(tc.tile_pool(name="xw", bufs=16))
        ptp = ctx.enter_context(tc.tile_pool(name="ptp", bufs=4))
        smalls = ctx.enter_context(tc.tile_pool(name="smalls", bufs=4))
        ydr = ctx.enter_context(tc.tile_pool(name="ydr", bufs=4))
        ps = ctx.enter_context(tc.tile_pool(name="ps", bufs=4, space="PSUM"))
        ps2 = ctx.enter_context(tc.tile_pool(name="ps2", bufs=2, space="PSUM"))

        # ---- constants / weights to SBUF ----
        w_qk_sb = wpool.tile([128, NKC, QKF], r32)
        nc.sync.dma_start(out=w_qk_sb, in_=w_qk.rearrange("(kc p) f -> p kc f", p=128))
        w_v_sb = wpool.tile([128, NKC, FW], r32)
        nc.sync.dma_start(out=w_v_sb, in_=w_v.rearrange("(kc p) f -> p kc f", p=128))
        w_p_sb = wpool.tile([128, 2, C], r32)
        nc.sync.dma_start(out=w_p_sb, in_=w_p.rearrange("(fc p) o -> p fc o", p=128))
        b_qk_sb = wpool.tile([128, 4], f32)
        nc.sync.dma_start(out=b_qk_sb, in_=b_qk.rearrange("(f p) -> p f", p=128))
        b_v_sb = wpool.tile([1, FW], r32)
        nc.sync.dma_start(out=b_v_sb, in_=b_v[None, :])
        masks_sb = wpool.tile([128, 4, 512], r32)
        nc.sync.dma_start(out=masks_sb, in_=masks.rearrange("j p q -> p j q"))
        ones_f = wpool.tile([1, 128], f32)
        nc.vector.memset(ones_f, 1.0)
        ones_sb = wpool.tile([1, 128], r32)
        nc.vector.tensor_copy(ones_sb, ones_f)
        onecol_f = wpool.tile([128, 1], f32)
        nc.vector.memset(onecol_f, 1.0)

        qkT_sb = big.tile([128, 4, T], r32)          # chunks: q01 q23 k01 k23
        v_sb = big.tile([128, NTT, HPC * (D + 1)], r32)
        z_sb = big.tile([128, 2, T], r32)

        # ---- stage A: qkv projection ----
        for tw in range(NTW):
            xts = []
            for kc in range(NKC):
                xt = xw.tile([128, 512], r32, tag="xt", name=f"xt_{tw}_{kc}")
                nc.sync.dma_start(
                    out=xt, in_=xT[kc * 128:(kc + 1) * 128, tw * 512:(tw + 1) * 512]
                )
                xts.append(xt)
            for f in range(4):
                ps_qk = ps.tile([128, 512], f32, tag="ps", name=f"psqk_{tw}_{f}")
                for kc in range(NKC):
                    nc.tensor.matmul(
                        ps_qk,
                        lhsT=w_qk_sb[:, kc, f * 128:(f + 1) * 128],
                        rhs=xts[kc],
                        start=(kc == 0),
                        stop=(kc == NKC - 1),
                    )
                nc.scalar.activation(
                    out=qkT_sb[:, f, tw * 512:(tw + 1) * 512],
                    in_=ps_qk,
                    func=AF.Identity,
                    bias=b_qk_sb[:, f:f + 1],
                    scale=1.0,
                )
            for t4 in range(4):
                tt = tw * 4 + t4
                ps_v = ps2.tile([128, FW], f32, tag="psv", name=f"psv_{tt}")
                for kc in range(NKC):
                    nc.tensor.matmul(
                        ps_v,
                        lhsT=xts[kc][:, t4 * 128:(t4 + 1) * 128],
                        rhs=w_v_sb[:, kc, :],
                        start=(kc == 0),
                        stop=False,
                    )
                nc.tensor.matmul(
                    ps_v,
                    lhsT=ones_sb,
                    rhs=b_v_sb,
                    start=False,
                    stop=True,
                )
                ones_cols = v_sb[:, tt, :].rearrange("p (h e) -> p h e", e=D + 1)[:, :, D]
                nc.vector.tensor_copy(ones_cols, onecol_f.broadcast_to([128, HPC]))
                for h in range(HPC):
                    nc.vector.tensor_copy(
                        v_sb[:, tt, h * (D + 1):h * (D + 1) + D],
                        ps_v[:, h * D:(h + 1) * D],
                    )

        # ---- stage B: attention (two heads packed per PE pass) ----
        for hp in range(2):
            for tw in range(NTW):
                nkc = 4 * (tw + 1)
                pvs = []
                for hh in range(2):
                    pv_t = ps2.tile([D + 1, 512], f32, tag="pv", name=f"pv_{hp}_{tw}_{hh}")
                    pvs.append(pv_t)
                for kc in range(nkc):
                    for hh in range(2):
                        h = 2 * hp + hh
                        base = 64 * (h % 2)
                        fq = h // 2
                        fk = 2 + h // 2
                        st = ps.tile([128, 512], f32, tag="ps", name=f"st_{hp}_{tw}_{kc}_{hh}")
                        nc.tensor.matmul(
                            st,
                            lhsT=qkT_sb[base:base + 64, fk, kc * 128:(kc + 1) * 128],
                            rhs=qkT_sb[base:base + 64, fq, tw * 512:(tw + 1) * 512],
                            start=True,
                            stop=True,
                        )
                        pt = ptp.tile([128, 512], r32, tag="pt", name=f"pt_{hp}_{tw}_{kc}_{hh}")
                        nc.scalar.activation(out=pt, in_=st, func=AF.Exp, scale=0.125)
                        j = kc - 4 * tw
                        if j >= 0:
                            nc.vector.tensor_mul(pt, pt, masks_sb[:, j, :])
                        nc.tensor.matmul(
                            pvs[hh],
                            lhsT=v_sb[:, kc, h * (D + 1):(h + 1) * (D + 1)],
                            rhs=pt,
                            start=(kc == 0),
                            stop=(kc == nkc - 1),
                        )
                for hh in range(2):
                    h = 2 * hp + hh
                    rec = smalls.tile([1, 512], r32, tag="rec", name=f"rec_{hp}_{tw}_{hh}")
                    nc.vector.reciprocal(rec, pvs[hh][D:D + 1, :])
                    Rps = ps2.tile([64, 512], f32, tag="psv", name=f"Rps_{hp}_{tw}_{hh}")
                    nc.tensor.matmul(
                        Rps,
                        lhsT=ones_sb[:, 0:64],
                        rhs=rec,
                        start=True,
                        stop=True,
                    )
                    Rb = smalls.tile([64, 512], r32, tag="Rb", name=f"Rb_{hp}_{tw}_{hh}")
                    nc.scalar.activation(out=Rb, in_=Rps, func=AF.Copy)
                    nc.vector.tensor_mul(
                        z_sb[64 * (h % 2):64 * (h % 2) + 64, h // 2, tw * 512:(tw + 1) * 512],
                        pvs[hh][0:D, :],
                        Rb,
                    )

        # ---- stage C: output projection (partial) ----
        for tt in range(NTT):
            for n in range(2):
                ps_y = ps.tile([128, 512], f32, tag="ps", name=f"psy_{tt}_{n}")
                for fc in range(2):
                    nc.tensor.matmul(
                        ps_y,
                        lhsT=z_sb[:, fc, tt * 128:(tt + 1) * 128],
                        rhs=w_p_sb[:, fc, n * 512:(n + 1) * 512],
                        start=(fc == 0),
                        stop=(fc == 1),
                    )
                yt = ydr.tile([128, 512], f32, tag="yt", name=f"yt_{tt}_{n}")
                nc.vector.tensor_copy(yt, ps_y)
                nc.sync.dma_start(
                    out=y[tt * 128:(tt + 1) * 128, n * 512:(n + 1) * 512], in_=yt
                )
    nc.finalize()
    return nc


def _causal_masks():
    j = np.arange(4)[:, None, None]
    p = np.arange(128)[None, :, None]
    q = np.arange(512)[None, None, :]
    return (q >= 128 * j + p).astype(np.float32)


def make_in_maps(x, W_attn, b_attn, W_proj):
    masks = _causal_masks()
    xT = [np.ascontiguousarray(x[b].T) for b in range(B)]
    in_maps = []
    for c in range(8):
        b, g = c // 4, c % 4
        heads = [4 * g + i for i in range(HPC)]
        wq = np.concatenate([W_attn[:, 64 * h:64 * h + 64] for h in heads], axis=1)
        wk = np.concatenate([W_attn[:, C + 64 * h:C + 64 * h + 64] for h in heads], axis=1)
        wv = np.concatenate([W_attn[:, 2 * C + 64 * h:2 * C + 64 * h + 64] for h in heads], axis=1)
        bq = np.concatenate([b_attn[64 * h:64 * h + 64] for h in heads])
        bk = np.concatenate([b_attn[C + 64 * h:C + 64 * h + 64] for h in heads])
        bv = np.concatenate([b_attn[2 * C + 64 * h:2 * C + 64 * h + 64] for h in heads])
        in_maps.append({
            "xT": xT[b],
            "w_qk": np.ascontiguousarray(np.concatenate([wq, wk], axis=1)),
            "b_qk": np.ascontiguousarray(np.concatenate([bq, bk])),
            "w_v": np.ascontiguousarray(wv),
            "b_v": np.ascontiguousarray(bv),
            "w_p": np.ascontiguousarray(W_proj[FW * g:FW * (g + 1), :]),
            "masks": masks,
        })
    return in_maps


def assemble(results, b_proj):
    y = np.zeros((B, T, C), np.float32)
    for c in range(8):
        y[c // 4] += results[c]["y"]
    y += b_proj[None, None, :].astype(np.float32)
    return y


def kernel(**inputs):
    from concourse.bass_utils import run_bass_kernel_spmd

    x = np.asarray(inputs["x"], np.float32)
    W_attn = np.asarray(inputs["W_attn"], np.float32)
    b_attn = np.asarray(inputs["b_attn"], np.float32)
    W_proj = np.asarray(inputs["W_proj"], np.float32)
    b_proj = np.asarray(inputs["b_proj"], np.float32)

    if "nc" not in _CACHE:
        _CACHE["nc"] = _build_nc()
    nc = _CACHE["nc"]
    in_maps = make_in_maps(x, W_attn, b_attn, W_proj)
    res = run_bass_kernel_spmd(nc, in_maps, core_ids=list(range(8))).results
    return assemble(res, b_proj)


# revision 16
# speedup vs baseline: 1.1287x; 1.1287x over previous
"""Causal self-attention on 8 trn2 NeuronCores.

Sharding: core c handles batch b = c // 4 and head group g = c % 4
(heads 4g..4g+3 of 16).  Each core computes:
  stage A: qkT = (W_qk_slice)^T @ x^T   (feature-major, d-major q/k)
           v   = x @ W_v_slice          (token-major, + ones column)
  stage B: per head, causal attention in S^T layout (keys on partitions,
           q on free dim): S^T = k @ q^T, P = exp(S/8) * mask,
           pv = [v | 1]^T @ P^T  -> rows 0..63 = out^T, row 64 = denom
           z = out^T / denom  (feature-major attention output)
  stage C: y_partial = z^T @ W_proj[row slice]   (token-major)
Host sums the 4 partials per batch and adds b_proj.

Matmul operands are bf16 (fp32 PSUM accumulation); the softmax
denominator reciprocal is kept at fp32/fp32r precision.
"""

import numpy as np

B, T, C, H, D = 2, 2048, 1024, 16, 64
HPC = 4              # heads per core
FW = HPC * D         # 256 attention-output features per core
QKF = 2 * FW         # 512 q+k features per core
NTW = T // 512       # 4 q/token windows of 512
NTT = T // 128       # 16 token tiles of 128
NKC = C // 128       # 8 contraction chunks for stage A

_CACHE = {}


def _build_nc(debug_outputs=False):
    import concourse.bass as bass  # noqa: F401
    import concourse.mybir as mybir
    import concourse.tile as tile
    from concourse import bacc
    from contextlib import ExitStack

    f32 = mybir.dt.float32
    r32 = mybir.dt.float32r
    bf16 = mybir.dt.bfloat16
    AF = mybir.ActivationFunctionType

    nc = bacc.Bacc(None, target_bir_lowering=False)
    xT = nc.declare_dram_parameter("xT", [C, T], bf16, isOutput=False)
    w_qk = nc.declare_dram_parameter("w_qk", [C, QKF], bf16, isOutput=False)
    b_qk = nc.declare_dram_parameter("b_qk", [QKF], f32, isOutput=False)
    w_v = nc.declare_dram_parameter("w_v", [C, FW], bf16, isOutput=False)
    b_v = nc.declare_dram_parameter("b_v", [FW], bf16, isOutput=False)
    w_p = nc.declare_dram_parameter("w_p", [FW, C], bf16, isOutput=False)
    masks = nc.declare_dram_parameter("masks", [4, 128, 512], bf16, isOutput=False)
    y = nc.declare_dram_parameter("y", [T, C], f32, isOutput=True)
    if debug_outputs:
        qkT_dbg = nc.declare_dram_parameter("qkT_dbg", [128, 4, T], bf16, isOutput=True)
        v_dbg = nc.declare_dram_parameter("v_dbg", [128, NTT, HPC * (D + 1)], bf16, isOutput=True)
        z_dbg = nc.declare_dram_parameter("z_dbg", [128, 2, T], bf16, isOutput=True)

    with nc.allow_low_precision(reason="bf16 matmul dataflow"), \
            tile.TileContext(nc) as tc, ExitStack() as ctx:
        wpool = ctx.enter_context(tc.tile_pool(name="wpool", bufs=1))
        big = ctx.enter_context(tc.tile_pool(name="big", bufs=1))
        xw = ctx.enter_context(tc.tile_pool(name="xw", bufs=16))
        ptp = ctx.enter_context(tc.tile_pool(name="ptp", bufs=6))
        smalls = ctx.enter_context(tc.tile_pool(name="smalls", bufs=4))
        ydr = ctx.enter_context(tc.tile_pool(name="ydr", bufs=4))
        ps = ctx.enter_context(tc.tile_pool(name="ps", bufs=3, space="PSUM"))
        psv = ctx.enter_context(tc.tile_pool(name="psv", bufs=1, space="PSUM"))
        ppv = ctx.enter_context(tc.tile_pool(name="ppv", bufs=4, space="PSUM"))

        # ---- constants / weights to SBUF ----
        w_qk_sb = wpool.tile([128, NKC, QKF], bf16)
        nc.sync.dma_start(out=w_qk_sb, in_=w_qk.rearrange("(kc p) f -> p kc f", p=128))
        w_v_sb = wpool.tile([128, NKC, FW], bf16)
        nc.sync.dma_start(out=w_v_sb, in_=w_v.rearrange("(kc p) f -> p kc f", p=128))
        w_p_sb = wpool.tile([128, 2, C], bf16)
        nc.sync.dma_start(out=w_p_sb, in_=w_p.rearrange("(fc p) o -> p fc o", p=128))
        b_qk_sb = wpool.tile([128, 4], f32)
        nc.sync.dma_start(out=b_qk_sb, in_=b_qk.rearrange("(f p) -> p f", p=128))
        b_v_sb = wpool.tile([1, FW], bf16)
        nc.sync.dma_start(out=b_v_sb, in_=b_v[None, :])
        masks_sb = wpool.tile([128, 4, 512], bf16)
        nc.sync.dma_start(out=masks_sb, in_=masks.rearrange("j p q -> p j q"))
        ones_f = wpool.tile([1, 128], f32)
        nc.vector.memset(ones_f, 1.0)
        ones_bf = wpool.tile([1, 128], bf16)
        nc.vector.tensor_copy(ones_bf, ones_f)
        ones_r = wpool.tile([1, 64], r32)
        nc.vector.tensor_copy(ones_r, ones_f[:, 0:64])
        onecol_f = wpool.tile([128, 1], f32)
        nc.vector.memset(onecol_f, 1.0)

        qkT_sb = big.tile([128, 4, T], bf16)         # chunks: q01 q23 k01 k23
        v_sb = big.tile([128, NTT, HPC * (D + 1)], bf16)
        z_sb = big.tile([128, 2, T], bf16)

        # ---- stage A: qkv projection ----
        for tw in range(NTW):
            xts = []
            for kc in range(NKC):
                xt = xw.tile([128, 512], bf16, tag="xt", name=f"xt_{tw}_{kc}")
                nc.sync.dma_start(
                    out=xt, in_=xT[kc * 128:(kc + 1) * 128, tw * 512:(tw + 1) * 512]
                )
                xts.append(xt)
            for f in range(4):
                ps_qk = ps.tile([128, 512], f32, tag="ps", name=f"psqk_{tw}_{f}")
                for kc in range(NKC):
                    nc.tensor.matmul(
                        ps_qk,
                        lhsT=w_qk_sb[:, kc, f * 128:(f + 1) * 128],
                        rhs=xts[kc],
                        start=(kc == 0),
                        stop=(kc == NKC - 1),
                    )
                nc.scalar.activation(
                    out=qkT_sb[:, f, tw * 512:(tw + 1) * 512],
                    in_=ps_qk,
                    func=AF.Identity,
                    bias=b_qk_sb[:, f:f + 1],
                    scale=1.0,
                )
            for t4 in range(4):
                tt = tw * 4 + t4
                ps_v = psv.tile([128, FW], f32, tag="psv", name=f"psv_{tt}")
                for kc in range(NKC):
                    nc.tensor.matmul(
                        ps_v,
                        lhsT=xts[kc][:, t4 * 128:(t4 + 1) * 128],
                        rhs=w_v_sb[:, kc, :],
                        start=(kc == 0),
                        stop=False,
                    )
                nc.tensor.matmul(
                    ps_v,
                    lhsT=ones_bf,
                    rhs=b_v_sb,
                    start=False,
                    stop=True,
                )
                ones_cols = v_sb[:, tt, :].rearrange("p (h e) -> p h e", e=D + 1)[:, :, D]
                nc.vector.tensor_copy(ones_cols, onecol_f.broadcast_to([128, HPC]))
                for h in range(HPC):
                    nc.vector.tensor_copy(
                        v_sb[:, tt, h * (D + 1):h * (D + 1) + D],
                        ps_v[:, h * D:(h + 1) * D],
                    )

        # ---- stage B: attention (two heads packed per PE pass) ----
        for hp in range(2):
            for tw in range(NTW):
                nkc = 4 * (tw + 1)
                pvs = []
                for hh in range(2):
                    pv_t = ppv.tile([D + 1, 512], f32, tag="pv", name=f"pv_{hp}_{tw}_{hh}")
                    pvs.append(pv_t)
                for kc in range(nkc):
                    for hh in range(2):
                        h = 2 * hp + hh
                        base = 64 * (h % 2)
                        fq = h // 2
                        fk = 2 + h // 2
                        st = ps.tile([128, 512], f32, tag="ps", name=f"st_{hp}_{tw}_{kc}_{hh}")
                        nc.tensor.matmul(
                            st,
                            lhsT=qkT_sb[base:base + 64, fk, kc * 128:(kc + 1) * 128],
                            rhs=qkT_sb[base:base + 64, fq, tw * 512:(tw + 1) * 512],
                            start=True,
                            stop=True,
                        )
                        pt = ptp.tile([128, 512], bf16, tag="pt", name=f"pt_{hp}_{tw}_{kc}_{hh}")
                        nc.scalar.activation(out=pt, in_=st, func=AF.Exp, scale=0.125)
                        j = kc - 4 * tw
                        if j >= 0:
                            nc.vector.tensor_mul(pt, pt, masks_sb[:, j, :])
                        nc.tensor.matmul(
                            pvs[hh],
                            lhsT=v_sb[:, kc, h * (D + 1):(h + 1) * (D + 1)],
                            rhs=pt,
                            start=(kc == 0),
                            stop=(kc == nkc - 1),
                        )
                for hh in range(2):
                    h = 2 * hp + hh
                    lnd = smalls.tile([1, 512], f32, tag="lnd", name=f"lnd_{hp}_{tw}_{hh}")
                    nc.scalar.activation(out=lnd, in_=pvs[hh][D:D + 1, :], func=AF.Ln)
                    rec_r = smalls.tile([1, 512], r32, tag="recr", name=f"recr_{hp}_{tw}_{hh}")
                    nc.scalar.activation(out=rec_r, in_=lnd, func=AF.Exp, scale=-1.0)
                    Rps = psv.tile([64, 512], f32, tag="psv", name=f"Rps_{hp}_{tw}_{hh}")
                    nc.tensor.matmul(Rps, lhsT=ones_r, rhs=rec_r, start=True, stop=True)
                    Rb = smalls.tile([64, 512], f32, tag="Rb", name=f"Rb_{hp}_{tw}_{hh}")
                    nc.scalar.activation(out=Rb, in_=Rps, func=AF.Copy)
                    nc.vector.tensor_mul(
                        z_sb[64 * (h % 2):64 * (h % 2) + 64, h // 2, tw * 512:(tw + 1) * 512],
                        pvs[hh][0:D, :],
                        Rb,
                    )

        # ---- stage C: output projection (partial) ----
        for tt in range(NTT):
            for n in range(2):
                ps_y = ps.tile([128, 512], f32, tag="ps", name=f"psy_{tt}_{n}")
                for fc in range(2):
                    nc.tensor.matmul(
                        ps_y,
                        lhsT=z_sb[:, fc, tt * 128:(tt + 1) * 128],
                        rhs=w_p_sb[:, fc, n * 512:(n + 1) * 512],
                        start=(fc == 0),
                        stop=(fc == 1),
                    )
                yt = ydr.tile([128, 512], f32, tag="yt", name=f"yt_{tt}_{n}")
                nc.vector.tensor_copy(yt, ps_y)
                nc.sync.dma_start(
                    out=y[tt * 128:(tt + 1) * 128, n * 512:(n + 1) * 512], in_=yt
                )

        if debug_outputs:
            nc.sync.dma_start(out=qkT_dbg[:, :, :], in_=qkT_sb)
            nc.sync.dma_start(out=v_dbg[:, :, :], in_=v_sb)
            nc.sync.dma_start(out=z_dbg[:, :, :], in_=z_sb)
    nc.finalize()
    return nc


def _causal_masks():
    j = np.arange(4)[:, None, None]
    p = np.arange(128)[None, :, None]
    q = np.arange(512)[None, None, :]
    return (q >= 128 * j + p).astype(np.float32)


def make_in_maps(x, W_attn, b_attn, W_proj):
    import ml_dtypes

    bf = ml_dtypes.bfloat16
    masks = _causal_masks().astype(bf)
    xT = [np.ascontiguousarray(x[b].T).astype(bf) for b in range(B)]
    in_maps = []
    for c in range(8):
        b, g = c // 4, c % 4
        heads = [4 * g + i for i in range(HPC)]
        wq = np.concatenate([W_attn[:, 64 * h:64 * h + 64] for h in heads], axis=1)
        wk = np.concatenate([W_attn[:, C + 64 * h:C + 64 * h + 64] for h in heads], axis=1)
        wv = np.concatenate([W_attn[:, 2 * C + 64 * h:2 * C + 64 * h + 64] for h in heads], axis=1)
        bq = np.concatenate([b_attn[64 * h:64 * h + 64] for h in heads])
        bk = np.concatenate([b_attn[C + 64 * h:C + 64 * h + 64] for h in heads])
        bv = np.concatenate([b_attn[2 * C + 64 * h:2 * C + 64 * h + 64] for h in heads])
        in_maps.append({
            "xT": xT[b],
            "w_qk": np.ascontiguousarray(np.concatenate([wq, wk], axis=1)).astype(bf),
            "b_qk": np.ascontiguousarray(np.concatenate([bq, bk]), dtype=np.float32),
            "w_v": np.ascontiguousarray(wv).astype(bf),
            "b_v": np.ascontiguousarray(bv).astype(bf),
            "w_p": np.ascontiguousarray(W_proj[FW * g:FW * (g + 1), :]).astype(bf),
            "masks": masks,
        })
    return in_maps


def assemble(results, b_proj):
    y = np.zeros((B, T, C), np.float32)
    for c in range(8):
        y[c // 4] += results[c]["y"]
    y += b_proj[None, None, :].astype(np.float32)
    return y


def kernel(**inputs):
    from concourse.bass_utils import run_bass_kernel_spmd

    x = np.asarray(inputs["x"], np.float32)
    W_attn = np.asarray(inputs["W_attn"], np.float32)
    b_attn = np.asarray(inputs["b_attn"], np.float32)
    W_proj = np.asarray(inputs["W_proj"], np.float32)
    b_proj = np.asarray(inputs["b_proj"], np.float32)

    if "nc" not in _CACHE:
        _CACHE["nc"] = _build_nc()
    nc = _CACHE["nc"]
    in_maps = make_in_maps(x, W_attn, b_attn, W_proj)
    res = run_bass_kernel_spmd(nc, in_maps, core_ids=list(range(8))).results
    return assemble(res, b_proj)


# revision 17
# speedup vs baseline: 1.2221x; 1.0828x over previous
"""Causal self-attention on 8 trn2 NeuronCores.

Sharding: core c handles batch b = c // 4 and head group g = c % 4
(heads 4g..4g+3 of 16).  Each core computes:
  stage A: qkT = (W_qk_slice)^T @ x^T   (feature-major, d-major q/k)
           v   = x @ W_v_slice          (token-major, + ones column)
  stage B: per head, causal attention in S^T layout (keys on partitions,
           q on free dim): S^T = k @ q^T, P = exp(S/8) * mask,
           pv = [v | 1]^T @ P^T  -> rows 0..63 = out^T, row 64 = denom
           z = out^T / denom  (feature-major attention output)
  stage C: y_partial = z^T @ W_proj[row slice]   (token-major)
Host sums the 4 partials per batch and adds b_proj.

Matmul operands are bf16 (fp32 PSUM accumulation); the softmax
denominator reciprocal is kept at fp32/fp32r precision.
"""

import numpy as np

B, T, C, H, D = 2, 2048, 1024, 16, 64
HPC = 4              # heads per core
FW = HPC * D         # 256 attention-output features per core
QKF = 2 * FW         # 512 q+k features per core
NTW = T // 512       # 4 q/token windows of 512
NTT = T // 128       # 16 token tiles of 128
NKC = C // 128       # 8 contraction chunks for stage A

_CACHE = {}


def _build_nc(debug_outputs=False):
    import concourse.bass as bass  # noqa: F401
    import concourse.mybir as mybir
    import concourse.tile as tile
    from concourse import bacc
    from contextlib import ExitStack

    f32 = mybir.dt.float32
    r32 = mybir.dt.float32r
    bf16 = mybir.dt.bfloat16
    AF = mybir.ActivationFunctionType

    nc = bacc.Bacc(None, target_bir_lowering=False)
    xT = nc.declare_dram_parameter("xT", [C, T], bf16, isOutput=False)
    w_qk = nc.declare_dram_parameter("w_qk", [C, QKF], bf16, isOutput=False)
    b_qk = nc.declare_dram_parameter("b_qk", [QKF], f32, isOutput=False)
    w_v = nc.declare_dram_parameter("w_v", [C, FW], bf16, isOutput=False)
    b_v = nc.declare_dram_parameter("b_v", [FW], bf16, isOutput=False)
    w_p = nc.declare_dram_parameter("w_p", [FW, C], bf16, isOutput=False)
    masks = nc.declare_dram_parameter("masks", [4, 128, 512], bf16, isOutput=False)
    y = nc.declare_dram_parameter("y", [T, C], f32, isOutput=True)
    if debug_outputs:
        qkT_dbg = nc.declare_dram_parameter("qkT_dbg", [128, 4, T], bf16, isOutput=True)
        v_dbg = nc.declare_dram_parameter("v_dbg", [128, NTT, HPC * (D + 1)], bf16, isOutput=True)
        z_dbg = nc.declare_dram_parameter("z_dbg", [128, 2, T], bf16, isOutput=True)

    with nc.allow_low_precision(reason="bf16 matmul dataflow"), \
            tile.TileContext(nc) as tc, ExitStack() as ctx:
        wpool = ctx.enter_context(tc.tile_pool(name="wpool", bufs=1))
        big = ctx.enter_context(tc.tile_pool(name="big", bufs=1))
        xw = ctx.enter_context(tc.tile_pool(name="xw", bufs=16))
        ptp = ctx.enter_context(tc.tile_pool(name="ptp", bufs=6))
        smalls = ctx.enter_context(tc.tile_pool(name="smalls", bufs=4))
        ydr = ctx.enter_context(tc.tile_pool(name="ydr", bufs=4))
        ps = ctx.enter_context(tc.tile_pool(name="ps", bufs=3, space="PSUM"))
        psv = ctx.enter_context(tc.tile_pool(name="psv", bufs=1, space="PSUM"))
        ppv = ctx.enter_context(tc.tile_pool(name="ppv", bufs=4, space="PSUM"))

        # ---- constants / weights to SBUF ----
        w_qk_sb = wpool.tile([128, NKC, QKF], bf16)
        nc.sync.dma_start(out=w_qk_sb, in_=w_qk.rearrange("(kc p) f -> p kc f", p=128))
        w_v_sb = wpool.tile([128, NKC, FW], bf16)
        nc.sync.dma_start(out=w_v_sb, in_=w_v.rearrange("(kc p) f -> p kc f", p=128))
        w_p_sb = wpool.tile([128, 2, C], bf16)
        nc.sync.dma_start(out=w_p_sb, in_=w_p.rearrange("(fc p) o -> p fc o", p=128))
        b_qk_sb = wpool.tile([128, 4], f32)
        nc.sync.dma_start(out=b_qk_sb, in_=b_qk.rearrange("(f p) -> p f", p=128))
        b_v_sb = wpool.tile([1, FW], bf16)
        nc.sync.dma_start(out=b_v_sb, in_=b_v[None, :])
        masks_sb = wpool.tile([128, 4, 512], bf16)
        nc.sync.dma_start(out=masks_sb, in_=masks.rearrange("j p q -> p j q"))
        ones_f = wpool.tile([1, 128], f32)
        nc.vector.memset(ones_f, 1.0)
        ones_bf = wpool.tile([1, 128], bf16)
        nc.vector.tensor_copy(ones_bf, ones_f)
        ones_r = wpool.tile([1, 64], r32)
        nc.vector.tensor_copy(ones_r, ones_f[:, 0:64])
        onecol_f = wpool.tile([128, 1], f32)
        nc.vector.memset(onecol_f, 1.0)

        qkT_sb = big.tile([128, 4, T], bf16)         # chunks: q01 q23 k01 k23
        v_sb = big.tile([128, NTT, HPC * (D + 1)], bf16)
        z_sb = big.tile([128, 2, T], bf16)

        # ---- interleaved per token-window: A(tw), B(tw), C(tw) ----
        def stage_a(tw):
            xts = []
            for kc in range(NKC):
                xt = xw.tile([128, 512], bf16, tag="xt", name=f"xt_{tw}_{kc}")
                nc.sync.dma_start(
                    out=xt, in_=xT[kc * 128:(kc + 1) * 128, tw * 512:(tw + 1) * 512]
                )
                xts.append(xt)
            for f in range(4):
                ps_qk = ps.tile([128, 512], f32, tag="ps", name=f"psqk_{tw}_{f}")
                for kc in range(NKC):
                    nc.tensor.matmul(
                        ps_qk,
                        lhsT=w_qk_sb[:, kc, f * 128:(f + 1) * 128],
                        rhs=xts[kc],
                        start=(kc == 0),
                        stop=(kc == NKC - 1),
                    )
                nc.scalar.activation(
                    out=qkT_sb[:, f, tw * 512:(tw + 1) * 512],
                    in_=ps_qk,
                    func=AF.Identity,
                    bias=b_qk_sb[:, f:f + 1],
                    scale=1.0,
                )
            for t4 in range(4):
                tt = tw * 4 + t4
                ps_v = psv.tile([128, FW], f32, tag="psv", name=f"psv_{tt}")
                for kc in range(NKC):
                    nc.tensor.matmul(
                        ps_v,
                        lhsT=xts[kc][:, t4 * 128:(t4 + 1) * 128],
                        rhs=w_v_sb[:, kc, :],
                        start=(kc == 0),
                        stop=False,
                    )
                nc.tensor.matmul(
                    ps_v,
                    lhsT=ones_bf,
                    rhs=b_v_sb,
                    start=False,
                    stop=True,
                )
                ones_cols = v_sb[:, tt, :].rearrange("p (h e) -> p h e", e=D + 1)[:, :, D]
                nc.vector.tensor_copy(ones_cols, onecol_f.broadcast_to([128, HPC]))
                for h in range(HPC):
                    nc.vector.tensor_copy(
                        v_sb[:, tt, h * (D + 1):h * (D + 1) + D],
                        ps_v[:, h * D:(h + 1) * D],
                    )

        def stage_b(hp, tw):
                nkc = 4 * (tw + 1)
                pvs = []
                for hh in range(2):
                    pv_t = ppv.tile([D + 1, 512], f32, tag="pv", name=f"pv_{hp}_{tw}_{hh}")
                    pvs.append(pv_t)
                for kc in range(nkc):
                    for hh in range(2):
                        h = 2 * hp + hh
                        base = 64 * (h % 2)
                        fq = h // 2
                        fk = 2 + h // 2
                        st = ps.tile([128, 512], f32, tag="ps", name=f"st_{hp}_{tw}_{kc}_{hh}")
                        nc.tensor.matmul(
                            st,
                            lhsT=qkT_sb[base:base + 64, fk, kc * 128:(kc + 1) * 128],
                            rhs=qkT_sb[base:base + 64, fq, tw * 512:(tw + 1) * 512],
                            start=True,
                            stop=True,
                        )
                        pt = ptp.tile([128, 512], bf16, tag="pt", name=f"pt_{hp}_{tw}_{kc}_{hh}")
                        nc.scalar.activation(out=pt, in_=st, func=AF.Exp, scale=0.125)
                        j = kc - 4 * tw
                        if j >= 0:
                            nc.vector.tensor_mul(pt, pt, masks_sb[:, j, :])
                        nc.tensor.matmul(
                            pvs[hh],
                            lhsT=v_sb[:, kc, h * (D + 1):(h + 1) * (D + 1)],
                            rhs=pt,
                            start=(kc == 0),
                            stop=(kc == nkc - 1),
                        )
                for hh in range(2):
                    h = 2 * hp + hh
                    lnd = smalls.tile([1, 512], f32, tag="lnd", name=f"lnd_{hp}_{tw}_{hh}")
                    nc.scalar.activation(out=lnd, in_=pvs[hh][D:D + 1, :], func=AF.Ln)
                    rec_r = smalls.tile([1, 512], r32, tag="recr", name=f"recr_{hp}_{tw}_{hh}")
                    nc.scalar.activation(out=rec_r, in_=lnd, func=AF.Exp, scale=-1.0)
                    Rps = psv.tile([64, 512], f32, tag="psv", name=f"Rps_{hp}_{tw}_{hh}")
                    nc.tensor.matmul(Rps, lhsT=ones_r, rhs=rec_r, start=True, stop=True)
                    Rb = smalls.tile([64, 512], f32, tag="Rb", name=f"Rb_{hp}_{tw}_{hh}")
                    nc.scalar.activation(out=Rb, in_=Rps, func=AF.Copy)
                    nc.vector.tensor_mul(
                        z_sb[64 * (h % 2):64 * (h % 2) + 64, h // 2, tw * 512:(tw + 1) * 512],
                        pvs[hh][0:D, :],
                        Rb,
                    )

        def stage_c(tt):
            for n in range(2):
                ps_y = ps.tile([128, 512], f32, tag="ps", name=f"psy_{tt}_{n}")
                for fc in range(2):
                    nc.tensor.matmul(
                        ps_y,
                        lhsT=z_sb[:, fc, tt * 128:(tt + 1) * 128],
                        rhs=w_p_sb[:, fc, n * 512:(n + 1) * 512],
                        start=(fc == 0),
                        stop=(fc == 1),
                    )
                yt = ydr.tile([128, 512], f32, tag="yt", name=f"yt_{tt}_{n}")
                nc.vector.tensor_copy(yt, ps_y)
                nc.sync.dma_start(
                    out=y[tt * 128:(tt + 1) * 128, n * 512:(n + 1) * 512], in_=yt
                )

        for tw in range(NTW):
            stage_a(tw)
            for hp in range(2):
                stage_b(hp, tw)
            for t4 in range(4):
                stage_c(tw * 4 + t4)

        if debug_outputs:
            nc.sync.dma_start(out=qkT_dbg[:, :, :], in_=qkT_sb)
            nc.sync.dma_start(out=v_dbg[:, :, :], in_=v_sb)
            nc.sync.dma_start(out=z_dbg[:, :, :], in_=z_sb)
    nc.finalize()
    return nc


def _causal_masks():
    j = np.arange(4)[:, None, None]
    p = np.arange(128)[None, :, None]
    q = np.arange(512)[None, None, :]
    return (q >= 128 * j + p).astype(np.float32)


def make_in_maps(x, W_attn, b_attn, W_proj):
    import ml_dtypes

    bf = ml_dtypes.bfloat16
    masks = _causal_masks().astype(bf)
    xT = [np.ascontiguousarray(x[b].T).astype(bf) for b in range(B)]
    in_maps = []
    for c in range(8):
        b, g = c // 4, c % 4
        heads = [4 * g + i for i in range(HPC)]
        wq = np.concatenate([W_attn[:, 64 * h:64 * h + 64] for h in heads], axis=1)
        wk = np.concatenate([W_attn[:, C + 64 * h:C + 64 * h + 64] for h in heads], axis=1)
        wv = np.concatenate([W_attn[:, 2 * C + 64 * h:2 * C + 64 * h + 64] for h in heads], axis=1)
        bq = np.concatenate([b_attn[64 * h:64 * h + 64] for h in heads])
        bk = np.concatenate([b_attn[C + 64 * h:C + 64 * h + 64] for h in heads])
        bv = np.concatenate([b_attn[2 * C + 64 * h:2 * C + 64 * h + 64] for h in heads])
        in_maps.append({
            "xT": xT[b],
            "w_qk": np.ascontiguousarray(np.concatenate([wq, wk], axis=1)).astype(bf),
            "b_qk": np.ascontiguousarray(np.concatenate([bq, bk]), dtype=np.float32),
            "w_v": np.ascontiguousarray(wv).astype(bf),
            "b_v": np.ascontiguousarray(bv).astype(bf),
            "w_p": np.ascontiguousarray(W_proj[FW * g:FW * (g + 1), :]).astype(bf),
            "masks": masks,
        })
    return in_maps


def assemble(results, b_proj):
    y = np.zeros((B, T, C), np.float32)
    for c in range(8):
        y[c // 4] += results[c]["y"]
    y += b_proj[None, None, :].astype(np.float32)
    return y


def kernel(**inputs):
    from concourse.bass_utils import run_bass_kernel_spmd

    x = np.asarray(inputs["x"], np.float32)
    W_attn = np.asarray(inputs["W_attn"], np.float32)
    b_attn = np.asarray(inputs["b_attn"], np.float32)
    W_proj = np.asarray(inputs["W_proj"], np.float32)
    b_proj = np.asarray(inputs["b_proj"], np.float32)

    if "nc" not in _CACHE:
        _CACHE["nc"] = _build_nc()
    nc = _CACHE["nc"]
    in_maps = make_in_maps(x, W_attn, b_attn, W_proj)
    res = run_bass_kernel_spmd(nc, in_maps, core_ids=list(range(8))).results
    return assemble(res, b_proj)


# revision 19
# speedup vs baseline: 1.3664x; 1.1181x over previous
"""Causal self-attention on 8 trn2 NeuronCores.

Sharding: core c handles batch b = c // 4 and head group g = c % 4
(heads 4g..4g+3 of 16).  Each core computes:
  stage A: qkT = (W_qk_slice)^T @ x^T   (feature-major, d-major q/k)
           v   = x @ W_v_slice          (token-major, + ones column)
  stage B: per head, causal attention in S^T layout (keys on partitions,
           q on free dim): S^T = k @ q^T, P = exp(S/8) * mask,
           pv = [v | 1]^T @ P^T  -> rows 0..63 = out^T, row 64 = denom
           z = out^T / denom  (feature-major attention output)
  stage C: y_partial = z^T @ W_proj[row slice]   (token-major)
Host sums the 4 partials per batch and adds b_proj.

Matmul operands are bf16 (fp32 PSUM accumulation); the softmax
denominator reciprocal is kept at fp32/fp32r precision.
"""

import numpy as np

B, T, C, H, D = 2, 2048, 1024, 16, 64
HPC = 4              # heads per core
FW = HPC * D         # 256 attention-output features per core
QKF = 2 * FW         # 512 q+k features per core
NTW = T // 512       # 4 q/token windows of 512
NTT = T // 128       # 16 token tiles of 128
NKC = C // 128       # 8 contraction chunks for stage A

_CACHE = {}


def _build_nc(debug_outputs=False):
    import concourse.bass as bass  # noqa: F401
    import concourse.mybir as mybir
    import concourse.tile as tile
    from concourse import bacc
    from contextlib import ExitStack

    f32 = mybir.dt.float32
    r32 = mybir.dt.float32r
    bf16 = mybir.dt.bfloat16
    AF = mybir.ActivationFunctionType

    nc = bacc.Bacc(None, target_bir_lowering=False)
    xT = nc.declare_dram_parameter("xT", [C, T], bf16, isOutput=False)
    w_qk = nc.declare_dram_parameter("w_qk", [C, QKF], bf16, isOutput=False)
    b_qk = nc.declare_dram_parameter("b_qk", [QKF], f32, isOutput=False)
    w_v = nc.declare_dram_parameter("w_v", [C, FW], bf16, isOutput=False)
    b_v = nc.declare_dram_parameter("b_v", [FW], bf16, isOutput=False)
    w_p = nc.declare_dram_parameter("w_p", [FW, C], bf16, isOutput=False)
    masks = nc.declare_dram_parameter("masks", [4, 128, 512], bf16, isOutput=False)
    y = nc.declare_dram_parameter("y", [T, C], f32, isOutput=True)
    if debug_outputs:
        qkT_dbg = nc.declare_dram_parameter("qkT_dbg", [128, 4, T], bf16, isOutput=True)
        v_dbg = nc.declare_dram_parameter("v_dbg", [128, NTT, HPC * (D + 1)], bf16, isOutput=True)
        z_dbg = nc.declare_dram_parameter("z_dbg", [128, 2, T], bf16, isOutput=True)

    with nc.allow_low_precision(reason="bf16 matmul dataflow"), \
            tile.TileContext(nc) as tc, ExitStack() as ctx:
        wpool = ctx.enter_context(tc.tile_pool(name="wpool", bufs=1))
        big = ctx.enter_context(tc.tile_pool(name="big", bufs=1))
        xw = ctx.enter_context(tc.tile_pool(name="xw", bufs=16))
        ptp = ctx.enter_context(tc.tile_pool(name="ptp", bufs=6))
        smalls = ctx.enter_context(tc.tile_pool(name="smalls", bufs=4))
        ydr = ctx.enter_context(tc.tile_pool(name="ydr", bufs=4))
        ps = ctx.enter_context(tc.tile_pool(name="ps", bufs=3, space="PSUM"))
        psv = ctx.enter_context(tc.tile_pool(name="psv", bufs=1, space="PSUM"))
        ppv = ctx.enter_context(tc.tile_pool(name="ppv", bufs=4, space="PSUM"))

        # ---- constants / weights to SBUF ----
        w_qk_sb = wpool.tile([128, NKC, QKF], bf16)
        nc.sync.dma_start(out=w_qk_sb, in_=w_qk.rearrange("(kc p) f -> p kc f", p=128))
        w_v_sb = wpool.tile([128, NKC, FW], bf16)
        nc.sync.dma_start(out=w_v_sb, in_=w_v.rearrange("(kc p) f -> p kc f", p=128))
        w_p_sb = wpool.tile([128, 2, C], bf16)
        nc.sync.dma_start(out=w_p_sb, in_=w_p.rearrange("(fc p) o -> p fc o", p=128))
        b_qk_sb = wpool.tile([128, 4], f32)
        nc.sync.dma_start(out=b_qk_sb, in_=b_qk.rearrange("(f p) -> p f", p=128))
        b_v_sb = wpool.tile([1, FW], bf16)
        nc.sync.dma_start(out=b_v_sb, in_=b_v[None, :])
        masks_sb = wpool.tile([128, 4, 512], bf16)
        nc.sync.dma_start(out=masks_sb, in_=masks.rearrange("j p q -> p j q"))
        ones_f = wpool.tile([1, 128], f32)
        nc.vector.memset(ones_f, 1.0)
        ones_bf = wpool.tile([1, 128], bf16)
        nc.vector.tensor_copy(ones_bf, ones_f)
        onecol_f = wpool.tile([128, 1], f32)
        nc.vector.memset(onecol_f, 1.0)

        qkT_sb = big.tile([128, 4, T], bf16)         # chunks: q01 q23 k01 k23
        v_sb = big.tile([128, NTT, HPC * (D + 1)], bf16)
        z_sb = big.tile([128, 2, T], bf16)

        # ---- interleaved per token-window: A(tw), B(tw), C(tw) ----
        def stage_a(tw):
            xts = []
            for kc in range(NKC):
                xt = xw.tile([128, 512], bf16, tag="xt", name=f"xt_{tw}_{kc}")
                nc.sync.dma_start(
                    out=xt, in_=xT[kc * 128:(kc + 1) * 128, tw * 512:(tw + 1) * 512]
                )
                xts.append(xt)
            for f in range(4):
                ps_qk = ps.tile([128, 512], f32, tag="ps", name=f"psqk_{tw}_{f}")
                for kc in range(NKC):
                    nc.tensor.matmul(
                        ps_qk,
                        lhsT=w_qk_sb[:, kc, f * 128:(f + 1) * 128],
                        rhs=xts[kc],
                        start=(kc == 0),
                        stop=(kc == NKC - 1),
                    )
                nc.scalar.activation(
                    out=qkT_sb[:, f, tw * 512:(tw + 1) * 512],
                    in_=ps_qk,
                    func=AF.Identity,
                    bias=b_qk_sb[:, f:f + 1],
                    scale=1.0,
                )
            for t4 in range(4):
                tt = tw * 4 + t4
                ps_v = psv.tile([128, FW], f32, tag="psv", name=f"psv_{tt}")
                for kc in range(NKC):
                    nc.tensor.matmul(
                        ps_v,
                        lhsT=xts[kc][:, t4 * 128:(t4 + 1) * 128],
                        rhs=w_v_sb[:, kc, :],
                        start=(kc == 0),
                        stop=False,
                    )
                nc.tensor.matmul(
                    ps_v,
                    lhsT=ones_bf,
                    rhs=b_v_sb,
                    start=False,
                    stop=True,
                )
                ones_cols = v_sb[:, tt, :].rearrange("p (h e) -> p h e", e=D + 1)[:, :, D]
                nc.vector.tensor_copy(ones_cols, onecol_f.broadcast_to([128, HPC]))
                for h in range(HPC):
                    nc.vector.tensor_copy(
                        v_sb[:, tt, h * (D + 1):h * (D + 1) + D],
                        ps_v[:, h * D:(h + 1) * D],
                    )

        def stage_b(hp, tw):
                nkc = 4 * (tw + 1)
                pvs = []
                for hh in range(2):
                    pv_t = ppv.tile([D + 1, 512], f32, tag="pv", name=f"pv_{hp}_{tw}_{hh}")
                    pvs.append(pv_t)
                for kc in range(nkc):
                    for hh in range(2):
                        h = 2 * hp + hh
                        base = 64 * (h % 2)
                        fq = h // 2
                        fk = 2 + h // 2
                        st = ps.tile([128, 512], f32, tag="ps", name=f"st_{hp}_{tw}_{kc}_{hh}")
                        nc.tensor.matmul(
                            st,
                            lhsT=qkT_sb[base:base + 64, fk, kc * 128:(kc + 1) * 128],
                            rhs=qkT_sb[base:base + 64, fq, tw * 512:(tw + 1) * 512],
                            start=True,
                            stop=True,
                        )
                        pt = ptp.tile([128, 512], bf16, tag="pt", name=f"pt_{hp}_{tw}_{kc}_{hh}")
                        nc.scalar.activation(out=pt, in_=st, func=AF.Exp, scale=0.125)
                        j = kc - 4 * tw
                        if j >= 0:
                            nc.vector.tensor_mul(pt, pt, masks_sb[:, j, :])
                        nc.tensor.matmul(
                            pvs[hh],
                            lhsT=v_sb[:, kc, h * (D + 1):(h + 1) * (D + 1)],
                            rhs=pt,
                            start=(kc == 0),
                            stop=(kc == nkc - 1),
                        )
                for hh in range(2):
                    h = 2 * hp + hh
                    rec_f = smalls.tile([1, 512], f32, tag="recf", name=f"recf_{hp}_{tw}_{hh}")
                    nc.vector.reciprocal(rec_f, pvs[hh][D:D + 1, :])
                    Rb = smalls.tile([64, 512], f32, tag="Rb", name=f"Rb_{hp}_{tw}_{hh}")
                    nc.gpsimd.partition_broadcast(Rb, rec_f)
                    nc.vector.tensor_mul(
                        z_sb[64 * (h % 2):64 * (h % 2) + 64, h // 2, tw * 512:(tw + 1) * 512],
                        pvs[hh][0:D, :],
                        Rb,
                    )

        def stage_c(tt):
            for n in range(2):
                ps_y = ps.tile([128, 512], f32, tag="ps", name=f"psy_{tt}_{n}")
                for fc in range(2):
                    nc.tensor.matmul(
                        ps_y,
                        lhsT=z_sb[:, fc, tt * 128:(tt + 1) * 128],
                        rhs=w_p_sb[:, fc, n * 512:(n + 1) * 512],
                        start=(fc == 0),
                        stop=(fc == 1),
                    )
                yt = ydr.tile([128, 512], f32, tag="yt", name=f"yt_{tt}_{n}")
                nc.vector.tensor_copy(yt, ps_y)
                nc.sync.dma_start(
                    out=y[tt * 128:(tt + 1) * 128, n * 512:(n + 1) * 512], in_=yt
                )

        for tw in range(NTW):
            stage_a(tw)
            for hp in range(2):
                stage_b(hp, tw)
            for t4 in range(4):
                stage_c(tw * 4 + t4)

        if debug_outputs:
            nc.sync.dma_start(out=qkT_dbg[:, :, :], in_=qkT_sb)
            nc.sync.dma_start(out=v_dbg[:, :, :], in_=v_sb)
            nc.sync.dma_start(out=z_dbg[:, :, :], in_=z_sb)
    nc.finalize()
    return nc


def _causal_masks():
    j = np.arange(4)[:, None, None]
    p = np.arange(128)[None, :, None]
    q = np.arange(512)[None, None, :]
    return (q >= 128 * j + p).astype(np.float32)


def make_in_maps(x, W_attn, b_attn, W_proj):
    import ml_dtypes

    bf = ml_dtypes.bfloat16
    masks = _causal_masks().astype(bf)
    xT = [np.ascontiguousarray(x[b].T).astype(bf) for b in range(B)]
    in_maps = []
    for c in range(8):
        b, g = c // 4, c % 4
        heads = [4 * g + i for i in range(HPC)]
        wq = np.concatenate([W_attn[:, 64 * h:64 * h + 64] for h in heads], axis=1)
        wk = np.concatenate([W_attn[:, C + 64 * h:C + 64 * h + 64] for h in heads], axis=1)
        wv = np.concatenate([W_attn[:, 2 * C + 64 * h:2 * C + 64 * h + 64] for h in heads], axis=1)
        bq = np.concatenate([b_attn[64 * h:64 * h + 64] for h in heads])
        bk = np.concatenate([b_attn[C + 64 * h:C + 64 * h + 64] for h in heads])
        bv = np.concatenate([b_attn[2 * C + 64 * h:2 * C + 64 * h + 64] for h in heads])
        in_maps.append({
            "xT": xT[b],
            "w_qk": np.ascontiguousarray(np.concatenate([wq, wk], axis=1)).astype(bf),
            "b_qk": np.ascontiguousarray(np.concatenate([bq, bk]), dtype=np.float32),
            "w_v": np.ascontiguousarray(wv).astype(bf),
            "b_v": np.ascontiguousarray(bv).astype(bf),
            "w_p": np.ascontiguousarray(W_proj[FW * g:FW * (g + 1), :]).astype(bf),
            "masks": masks,
        })
    return in_maps


def assemble(results, b_proj):
    y = np.zeros((B, T, C), np.float32)
    for c in range(8):
        y[c // 4] += results[c]["y"]
    y += b_proj[None, None, :].astype(np.float32)
    return y


def kernel(**inputs):
    from concourse.bass_utils import run_bass_kernel_spmd

    x = np.asarray(inputs["x"], np.float32)
    W_attn = np.asarray(inputs["W_attn"], np.float32)
    b_attn = np.asarray(inputs["b_attn"], np.float32)
    W_proj = np.asarray(inputs["W_proj"], np.float32)
    b_proj = np.asarray(inputs["b_proj"], np.float32)

    if "nc" not in _CACHE:
        _CACHE["nc"] = _build_nc()
    nc = _CACHE["nc"]
    in_maps = make_in_maps(x, W_attn, b_attn, W_proj)
    res = run_bass_kernel_spmd(nc, in_maps, core_ids=list(range(8))).results
    return assemble(res, b_proj)


# revision 20
# speedup vs baseline: 1.3942x; 1.0203x over previous
"""Causal self-attention on 8 trn2 NeuronCores.

Sharding: core c handles batch b = c // 4 and head group g = c % 4
(heads 4g..4g+3 of 16).  Each core computes:
  stage A: qkT = (W_qk_slice)^T @ x^T   (feature-major, d-major q/k)
           v   = x @ W_v_slice          (token-major, + ones column)
  stage B: per head, causal attention in S^T layout (keys on partitions,
           q on free dim): S^T = k @ q^T, P = exp(S/8) * mask,
           pv = [v | 1]^T @ P^T  -> rows 0..63 = out^T, row 64 = denom
           z = out^T / denom  (feature-major attention output)
  stage C: y_partial = z^T @ W_proj[row slice]   (token-major)
Host sums the 4 partials per batch and adds b_proj.

Matmul operands are bf16 (fp32 PSUM accumulation); the softmax
denominator reciprocal is kept at fp32/fp32r precision.
"""

import numpy as np

B, T, C, H, D = 2, 2048, 1024, 16, 64
HPC = 4              # heads per core
FW = HPC * D         # 256 attention-output features per core
QKF = 2 * FW         # 512 q+k features per core
NTW = T // 512       # 4 q/token windows of 512
NTT = T // 128       # 16 token tiles of 128
NKC = C // 128       # 8 contraction chunks for stage A

_CACHE = {}


def _build_nc(debug_outputs=False):
    import concourse.bass as bass  # noqa: F401
    import concourse.mybir as mybir
    import concourse.tile as tile
    from concourse import bacc
    from contextlib import ExitStack

    f32 = mybir.dt.float32
    r32 = mybir.dt.float32r
    bf16 = mybir.dt.bfloat16
    AF = mybir.ActivationFunctionType

    nc = bacc.Bacc(None, target_bir_lowering=False)
    xT = nc.declare_dram_parameter("xT", [C, T], bf16, isOutput=False)
    w_qk = nc.declare_dram_parameter("w_qk", [C, QKF], bf16, isOutput=False)
    b_qk = nc.declare_dram_parameter("b_qk", [QKF], f32, isOutput=False)
    w_v = nc.declare_dram_parameter("w_v", [C, FW], bf16, isOutput=False)
    b_v = nc.declare_dram_parameter("b_v", [FW], bf16, isOutput=False)
    w_p = nc.declare_dram_parameter("w_p", [FW, C], bf16, isOutput=False)
    masks = nc.declare_dram_parameter("masks", [4, 128, 512], bf16, isOutput=False)
    y = nc.declare_dram_parameter("y", [T, C], f32, isOutput=True)
    if debug_outputs:
        qkT_dbg = nc.declare_dram_parameter("qkT_dbg", [128, 4, T], bf16, isOutput=True)
        v_dbg = nc.declare_dram_parameter("v_dbg", [128, NTT, HPC * 2 * D], bf16, isOutput=True)
        z_dbg = nc.declare_dram_parameter("z_dbg", [128, 2, T], bf16, isOutput=True)

    with nc.allow_low_precision(reason="bf16 matmul dataflow"), \
            tile.TileContext(nc) as tc, ExitStack() as ctx:
        wpool = ctx.enter_context(tc.tile_pool(name="wpool", bufs=1))
        big = ctx.enter_context(tc.tile_pool(name="big", bufs=1))
        xw = ctx.enter_context(tc.tile_pool(name="xw", bufs=16))
        ptp = ctx.enter_context(tc.tile_pool(name="ptp", bufs=6))
        smalls = ctx.enter_context(tc.tile_pool(name="smalls", bufs=4))
        ydr = ctx.enter_context(tc.tile_pool(name="ydr", bufs=4))
        ps = ctx.enter_context(tc.tile_pool(name="ps", bufs=3, space="PSUM"))
        psv = ctx.enter_context(tc.tile_pool(name="psv", bufs=1, space="PSUM"))
        ppv = ctx.enter_context(tc.tile_pool(name="ppv", bufs=4, space="PSUM"))

        # ---- constants / weights to SBUF ----
        w_qk_sb = wpool.tile([128, NKC, QKF], bf16)
        nc.sync.dma_start(out=w_qk_sb, in_=w_qk.rearrange("(kc p) f -> p kc f", p=128))
        w_v_sb = wpool.tile([128, NKC, FW], bf16)
        nc.sync.dma_start(out=w_v_sb, in_=w_v.rearrange("(kc p) f -> p kc f", p=128))
        w_p_sb = wpool.tile([128, 2, C], bf16)
        nc.sync.dma_start(out=w_p_sb, in_=w_p.rearrange("(fc p) o -> p fc o", p=128))
        b_qk_sb = wpool.tile([128, 4], f32)
        nc.sync.dma_start(out=b_qk_sb, in_=b_qk.rearrange("(f p) -> p f", p=128))
        b_v_sb = wpool.tile([1, FW], bf16)
        nc.sync.dma_start(out=b_v_sb, in_=b_v[None, :])
        masks_sb = wpool.tile([128, 4, 512], bf16)
        nc.sync.dma_start(out=masks_sb, in_=masks.rearrange("j p q -> p j q"))
        ones_f = wpool.tile([1, 128], f32)
        nc.vector.memset(ones_f, 1.0)
        ones_bf = wpool.tile([1, 128], bf16)
        nc.vector.tensor_copy(ones_bf, ones_f)
        onecol_f = wpool.tile([128, 1], f32)
        nc.vector.memset(onecol_f, 1.0)

        qkT_sb = big.tile([128, 4, T], bf16)         # chunks: q01 q23 k01 k23
        v_sb = big.tile([128, NTT, HPC * 2 * D], bf16)
        z_sb = big.tile([128, 2, T], bf16)

        # ---- interleaved per token-window: A(tw), B(tw), C(tw) ----
        def stage_a(tw):
            xts = []
            for kc in range(NKC):
                xt = xw.tile([128, 512], bf16, tag="xt", name=f"xt_{tw}_{kc}")
                nc.sync.dma_start(
                    out=xt, in_=xT[kc * 128:(kc + 1) * 128, tw * 512:(tw + 1) * 512]
                )
                xts.append(xt)
            for f in range(4):
                ps_qk = ps.tile([128, 512], f32, tag="ps", name=f"psqk_{tw}_{f}")
                for kc in range(NKC):
                    nc.tensor.matmul(
                        ps_qk,
                        lhsT=w_qk_sb[:, kc, f * 128:(f + 1) * 128],
                        rhs=xts[kc],
                        start=(kc == 0),
                        stop=(kc == NKC - 1),
                    )
                nc.scalar.activation(
                    out=qkT_sb[:, f, tw * 512:(tw + 1) * 512],
                    in_=ps_qk,
                    func=AF.Identity,
                    bias=b_qk_sb[:, f:f + 1],
                    scale=1.0,
                )
            for t4 in range(4):
                tt = tw * 4 + t4
                ps_v = psv.tile([128, FW], f32, tag="psv", name=f"psv_{tt}")
                for kc in range(NKC):
                    nc.tensor.matmul(
                        ps_v,
                        lhsT=xts[kc][:, t4 * 128:(t4 + 1) * 128],
                        rhs=w_v_sb[:, kc, :],
                        start=(kc == 0),
                        stop=False,
                    )
                nc.tensor.matmul(
                    ps_v,
                    lhsT=ones_bf,
                    rhs=b_v_sb,
                    start=False,
                    stop=True,
                )
                ones_cols = v_sb[:, tt, :].rearrange("p (h e) -> p h e", e=2 * D)[:, :, D:]
                nc.vector.tensor_copy(ones_cols, onecol_f[:, :, None].broadcast_to([128, HPC, D]))
                for h in range(HPC):
                    nc.vector.tensor_copy(
                        v_sb[:, tt, h * 2 * D:h * 2 * D + D],
                        ps_v[:, h * D:(h + 1) * D],
                    )

        def stage_b(hp, tw):
                nkc = 4 * (tw + 1)
                pvs = []
                for hh in range(2):
                    pv_t = ppv.tile([2 * D, 512], f32, tag="pv", name=f"pv_{hp}_{tw}_{hh}")
                    pvs.append(pv_t)
                for kc in range(nkc):
                    for hh in range(2):
                        h = 2 * hp + hh
                        base = 64 * (h % 2)
                        fq = h // 2
                        fk = 2 + h // 2
                        st = ps.tile([128, 512], f32, tag="ps", name=f"st_{hp}_{tw}_{kc}_{hh}")
                        nc.tensor.matmul(
                            st,
                            lhsT=qkT_sb[base:base + 64, fk, kc * 128:(kc + 1) * 128],
                            rhs=qkT_sb[base:base + 64, fq, tw * 512:(tw + 1) * 512],
                            start=True,
                            stop=True,
                        )
                        pt = ptp.tile([128, 512], bf16, tag="pt", name=f"pt_{hp}_{tw}_{kc}_{hh}")
                        nc.scalar.activation(out=pt, in_=st, func=AF.Exp, scale=0.125)
                        j = kc - 4 * tw
                        if j >= 0:
                            nc.vector.tensor_mul(pt, pt, masks_sb[:, j, :])
                        nc.tensor.matmul(
                            pvs[hh],
                            lhsT=v_sb[:, kc, h * 2 * D:(h + 1) * 2 * D],
                            rhs=pt,
                            start=(kc == 0),
                            stop=(kc == nkc - 1),
                        )
                for hh in range(2):
                    h = 2 * hp + hh
                    rec_f = smalls.tile([1, 512], f32, tag="recf", name=f"recf_{hp}_{tw}_{hh}")
                    nc.vector.reciprocal(rec_f, pvs[hh][D:D + 1, :])
                    Rb = smalls.tile([64, 512], f32, tag="Rb", name=f"Rb_{hp}_{tw}_{hh}")
                    nc.gpsimd.partition_broadcast(Rb, rec_f)
                    nc.vector.tensor_mul(
                        z_sb[64 * (h % 2):64 * (h % 2) + 64, h // 2, tw * 512:(tw + 1) * 512],
                        pvs[hh][0:D, :],
                        Rb,
                    )

        def stage_c(tt):
            for n in range(2):
                ps_y = ps.tile([128, 512], f32, tag="ps", name=f"psy_{tt}_{n}")
                for fc in range(2):
                    nc.tensor.matmul(
                        ps_y,
                        lhsT=z_sb[:, fc, tt * 128:(tt + 1) * 128],
                        rhs=w_p_sb[:, fc, n * 512:(n + 1) * 512],
                        start=(fc == 0),
                        stop=(fc == 1),
                    )
                yt = ydr.tile([128, 512], f32, tag="yt", name=f"yt_{tt}_{n}")
                nc.vector.tensor_copy(yt, ps_y)
                nc.sync.dma_start(
                    out=y[tt * 128:(tt + 1) * 128, n * 512:(n + 1) * 512], in_=yt
                )

        for tw in range(NTW):
            stage_a(tw)
            for hp in range(2):
                stage_b(hp, tw)
            if tw > 0:
                for t4 in range(4):
                    stage_c((tw - 1) * 4 + t4)
        for t4 in range(4):
            stage_c(3 * 4 + t4)

        if debug_outputs:
            nc.sync.dma_start(out=qkT_dbg[:, :, :], in_=qkT_sb)
            nc.sync.dma_start(out=v_dbg[:, :, :], in_=v_sb)
            nc.sync.dma_start(out=z_dbg[:, :, :], in_=z_sb)
    nc.finalize()
    return nc


def _causal_masks():
    j = np.arange(4)[:, None, None]
    p = np.arange(128)[None, :, None]
    q = np.arange(512)[None, None, :]
    return (q >= 128 * j + p).astype(np.float32)


def make_in_maps(x, W_attn, b_attn, W_proj):
    import ml_dtypes

    bf = ml_dtypes.bfloat16
    masks = _causal_masks().astype(bf)
    xT = [np.ascontiguousarray(x[b].T).astype(bf) for b in range(B)]
    in_maps = []
    for c in range(8):
        b, g = c // 4, c % 4
        heads = [4 * g + i for i in range(HPC)]
        wq = np.concatenate([W_attn[:, 64 * h:64 * h + 64] for h in heads], axis=1)
        wk = np.concatenate([W_attn[:, C + 64 * h:C + 64 * h + 64] for h in heads], axis=1)
        wv = np.concatenate([W_attn[:, 2 * C + 64 * h:2 * C + 64 * h + 64] for h in heads], axis=1)
        bq = np.concatenate([b_attn[64 * h:64 * h + 64] for h in heads])
        bk = np.concatenate([b_attn[C + 64 * h:C + 64 * h + 64] for h in heads])
        bv = np.concatenate([b_attn[2 * C + 64 * h:2 * C + 64 * h + 64] for h in heads])
        in_maps.append({
            "xT": xT[b],
            "w_qk": np.ascontiguousarray(np.concatenate([wq, wk], axis=1)).astype(bf),
            "b_qk": np.ascontiguousarray(np.concatenate([bq, bk]), dtype=np.float32),
            "w_v": np.ascontiguousarray(wv).astype(bf),
            "b_v": np.ascontiguousarray(bv).astype(bf),
            "w_p": np.ascontiguousarray(W_proj[FW * g:FW * (g + 1), :]).astype(bf),
            "masks": masks,
        })
    return in_maps


def assemble(results, b_proj):
    y = np.zeros((B, T, C), np.float32)
    for c in range(8):
        y[c // 4] += results[c]["y"]
    y += b_proj[None, None, :].astype(np.float32)
    return y


def kernel(**inputs):
    from concourse.bass_utils import run_bass_kernel_spmd

    x = np.asarray(inputs["x"], np.float32)
    W_attn = np.asarray(inputs["W_attn"], np.float32)
    b_attn = np.asarray(inputs["b_attn"], np.float32)
    W_proj = np.asarray(inputs["W_proj"], np.float32)
    b_proj = np.asarray(inputs["b_proj"], np.float32)

    if "nc" not in _CACHE:
        _CACHE["nc"] = _build_nc()
    nc = _CACHE["nc"]
    in_maps = make_in_maps(x, W_attn, b_attn, W_proj)
    res = run_bass_kernel_spmd(nc, in_maps, core_ids=list(range(8))).results
    return assemble(res, b_proj)
